# revision 1
# baseline (speedup 1.0000x reference)
"""Trainium2 Bass kernel for nn_ConditionalTimeGradPredictionNetwork.

Math (see reference): EWMA-scale student-t(df=4) Gaussianization of x_hist
plus inverse transform of z_future forecasts.

Design (per NeuronCore, data-parallel over batch, 16 batches/core):
  - layout [t-in-block(128) partitions, (block, batch, d) free]
  - EWMA variance scan as block-triangular matmuls on the TensorEngine in
    float32r: psum = c2 + 2*ewma directly ((2A+I) current block + 2B prev +
    2C prev-prev; truncation error ~alpha^384 ~ 5e-11)
  - loc (mean over t) on GPSIMD (tree add + partition_all_reduce)
  - ndtri(t4cdf(x)) composite: z = G(v)*s with s = xm*rsqrt(c4),
    v = sqrt(-ln(om)), om = h^2(3+h)/4, h = 1 - c2/c4; G = deg-7 poly
    (fit offline, fp32-exact 5e-6); custom fused DVE ops for the hot path.
  - forecast branch: erf/arctan/sin closed form with an erfc-tail path for
    |z| > 3 (fp32-cancellation-free complement).
"""
import numpy as np
from contextlib import ExitStack

# ---------------- constants (from offline fit) ----------------
ALPHA = 0.94
OMIN4 = 1.6e-5
KB = 128          # t-block size
NBLK = 16         # blocks (T = 2048)
T = 2048
P_FUT = 64
D = 128
B_TOT = 128
NCORES = 8
B_CORE = B_TOT // NCORES      # 16
GB = 2                        # batches per group
NG = B_CORE // GB             # 8 groups
FREE_G = NBLK * GB * D        # 4096
W_G = GB * D                  # 256

G_COEF = np.array([5.87717804e-04, -4.81543312e-03, 1.07848029e-02,
                   -4.09072944e-03, 6.36268591e-03, 2.13134753e-01,
                   -5.77367385e-04, 1.88008859e+00], dtype=np.float64)
S_COEF = np.array([-1.20060025e-05, 3.00739478e-04, -3.30522924e-03,
                   2.09304994e-02, -8.42975312e-02, 2.24654353e-01,
                   -3.98166530e-01, 4.52962591e-01], dtype=np.float64)
SQ12 = 0.7071067811865476


def _build_matrices():
    i = np.arange(KB)[:, None]
    j = np.arange(KB)[None, :]
    A = np.where(j <= i, (1 - ALPHA) * ALPHA ** np.clip(i - j, 0, None), 0.0)
    A0 = A.copy(); A0[:, 0] = ALPHA ** i[:, 0]
    B = (1 - ALPHA) * ALPHA ** (i + KB - j)
    B0 = B.copy(); B0[:, 0] = ALPHA ** (i[:, 0] + KB)
    C = (1 - ALPHA) * ALPHA ** (i + 2 * KB - j)
    C0 = C.copy(); C0[:, 0] = ALPHA ** (i[:, 0] + 2 * KB)
    I = np.eye(KB)
    mats = [2 * A + I, 2 * A0 + I, 2 * B, 2 * B0, 2 * C, 2 * C0]
    # lhsT layout [k=j, m=i] = M.T
    lhsTs = [m.T.astype(np.float32) for m in mats]
    # P_last broadcast weights: out[m,n] = sum_j W[j,m]*c2[j,n]; W[j,:] = w[j]
    wA = 2 * (1 - ALPHA) * ALPHA ** (KB - 1 - np.arange(KB))
    for scale in (1.0, ALPHA ** KB, ALPHA ** (2 * KB)):
        lhsTs.append(np.repeat((wA * scale)[:, None], KB, axis=1).astype(np.float32))
    return np.concatenate(lhsTs, axis=1)  # [128, 9*128]


# ---------------- custom DVE ops (runtime registered) ----------------
def _register_ops():
    from concourse import dve_ops
    from concourse.dve_spec import (
        Spec, Src0, Src1, C0, C1, C2, C3, One, maxx, sq, lower,
        _has_src1, _spill_c3_to_src1,
    )
    from concourse.dve_uop import DveOpSpec

    def reg(name, spec, subdim=False):
        for op in dve_ops.OPS:
            if op.name == name:
                return op
        row = max(dve_ops._SUB_OPCODE_FOR_NAME.values()) + 1
        assert row < 0x20
        dve_ops._SUB_OPCODE_FOR_NAME[name] = row
        shas = {}
        for ver in ("v3", "v4"):
            tmp = DveOpSpec(name=name, opcode=row, uops=lower(spec, ver=ver),
                            rd1_en=_has_src1(spec))
            shas[ver] = tmp.sha(ver)
        op = dve_ops.DveOp(name, spec, subdim=subdim, uops_sha=shas)
        dve_ops.OPS.append(op)
        dve_ops.CUSTOM_DVE_SPECS[name] = spec
        return op

    _h = One - Src0 * Src1
    omc = reg("OMC2_ANT", Spec(
        body=maxx(sq(_h) * (_h + C0), C1),
        reference=lambda in0, in1, s0, s1, imm2: np.maximum(
            (1.0 - in0 * in1) ** 2 * ((1.0 - in0 * in1) + s0), s1),
    ))
    _h4 = (((C0 * Src0 + C1) * Src0 + C2) * Src0 + C3) * Src0
    h4 = reg("HORNER4_ANT", Spec(
        body=_spill_c3_to_src1(_h4),
        reference=lambda in0, in1, s0, s1, imm2: (
            ((s0 * in0 + s1) * in0 + imm2) * in0 + in1) * in0,
    ))
    _h3s = (((Src0 + C0) * Src1 + C1) * Src1 + C2) * Src1
    h3s = reg("HORNER3S_ANT", Spec(
        body=_h3s,
        reference=lambda in0, in1, s0, s1, imm2: (
            ((in0 + s0) * in1 + s1) * in1 + imm2) * in1,
    ))
    _rnr = Src0 * (C0 - C1 * Src1 * sq(Src0))
    rnr = reg("RSQRT_NR_ANT", Spec(
        body=_rnr,
        reference=lambda in0, in1, s0, s1, imm2: in0 * (s0 - s1 * in1 * in0 * in0),
    ))
    return omc, h4, h3s, rnr


# ---------------- bass program ----------------
_CACHE = {}


def _build_program():
    if "nc" in _CACHE:
        return _CACHE["nc"]
    import concourse.bass as bass
    import concourse.tile as tile
    import concourse.bass_isa as bass_isa
    from concourse import bacc, mybir

    OMC_OP, H4_OP, H3S_OP, RNR_OP = _register_ops()

    F32 = mybir.dt.float32
    F32R = mybir.dt.float32r
    AT = mybir.ActivationFunctionType
    OP = mybir.AluOpType

    nc = bacc.Bacc("TRN2", target_bir_lowering=False, debug=False,
                   num_devices=NCORES)

    x_in = nc.dram_tensor("x", [B_CORE, T, D], F32, kind="ExternalInput").ap()
    zf_in = nc.dram_tensor("zf", [B_CORE, P_FUT, D], F32, kind="ExternalInput").ap()
    mats_in = nc.dram_tensor("mats", [KB, 9 * KB], F32, kind="ExternalInput").ap()
    out = nc.dram_tensor("out", [B_CORE, T + P_FUT, D], F32,
                         kind="ExternalOutput").ap()

    gc = [float(c) for c in G_COEF]
    sc = [float(c) for c in S_COEF]

    def fl(ap):
        # flatten 2 free dims -> 1 (custom-dve TTSS struct needs 1D src1)
        if len(ap.shape) == 3:
            return ap.rearrange("p a b -> p (a b)")
        return ap

    def cdve(op, o, in0, in1=None, s0=0.0, s1=0.0, imm2=0.0):
        return nc.vector._custom_dve(op, out=fl(o), in0=fl(in0),
                                     in1=None if in1 is None else fl(in1),
                                     s0=s0, s1=s1, imm2=imm2)

    with tile.TileContext(nc) as tc, ExitStack() as ctx:
        cpool = ctx.enter_context(tc.tile_pool(name="consts", bufs=1))
        xpool = ctx.enter_context(tc.tile_pool(name="x", bufs=2))
        mpool = ctx.enter_context(tc.tile_pool(name="mid", bufs=1))
        wpool1 = ctx.enter_context(tc.tile_pool(name="work1", bufs=1))
        wpool2 = ctx.enter_context(tc.tile_pool(name="work2", bufs=2))
        spool = ctx.enter_context(tc.tile_pool(name="small", bufs=2))
        lpool = ctx.enter_context(tc.tile_pool(name="locs", bufs=1))
        ppool = ctx.enter_context(tc.tile_pool(name="psum", bufs=3, space="PSUM"))
        pp2 = ctx.enter_context(tc.tile_pool(name="psum2", bufs=2, space="PSUM"))

        # constants
        mats_f = cpool.tile([KB, 9 * KB], F32)
        nc.sync.dma_start(mats_f[:], mats_in)
        mats_r = cpool.tile([KB, 9 * KB], F32R)
        nc.vector.tensor_copy(mats_r[:], mats_f[:])

        def lhsT(idx):
            return mats_r[:, idx * KB:(idx + 1) * KB]
        MA, MA0, MB, MB0, MC, MC0, WA, WB, WC = range(9)

        g3t = cpool.tile([KB, 1], F32)
        nc.vector.memset(g3t[:], gc[3])
        s3t = cpool.tile([KB, 1], F32)
        nc.vector.memset(s3t[:], sc[3])
        pio2 = cpool.tile([KB, 1], F32)
        nc.vector.memset(pio2[:], float(np.pi / 2))

        locs_tiles = []
        plb_tiles = []

        HB = NBLK // 2          # blocks per half-chain (8)
        WH = HB * W_G           # half free size

        for g in range(NG):
            b0 = g * GB
            x_g = xpool.tile([KB, NBLK, W_G], F32, tag="x_g")
            for b in range(GB):
                src = x_in[b0 + b].rearrange("(k p) d -> p k d", p=KB)
                nc.sync.dma_start(x_g[:, :, b * D:(b + 1) * D], src)

            # ---- loc on gpsimd: tree-sum over the 16 block-slices ----
            acc8 = wpool2.tile([KB, 8, W_G], F32, tag="acc8")
            for k in range(8):
                nc.gpsimd.tensor_tensor(acc8[:, k, :], x_g[:, 2 * k, :],
                                        x_g[:, 2 * k + 1, :], OP.add)
            acc4 = spool.tile([KB, 4, W_G], F32, tag="acc4")
            for k in range(4):
                nc.gpsimd.tensor_tensor(acc4[:, k, :], acc8[:, 2 * k, :],
                                        acc8[:, 2 * k + 1, :], OP.add)
            acc2 = spool.tile([KB, 2, W_G], F32, tag="acc2")
            for k in range(2):
                nc.gpsimd.tensor_tensor(acc2[:, k, :], acc4[:, 2 * k, :],
                                        acc4[:, 2 * k + 1, :], OP.add)
            acc1 = spool.tile([KB, W_G], F32, tag="acc1")
            nc.gpsimd.tensor_tensor(acc1[:], acc2[:, 0, :], acc2[:, 1, :], OP.add)
            locs = lpool.tile([KB, W_G], F32, tag=f"locs{g}")
            nc.gpsimd.partition_all_reduce(locs[:], acc1[:], KB,
                                           bass_isa.ReduceOp.add)
            locs_tiles.append(locs)

            # ---- xm = x - locs/T  (locs broadcast over blocks) ----
            xm = mpool.tile([KB, NBLK, W_G], F32, tag="xm")
            locs_b = locs[:].rearrange("p (o n) -> p o n", o=1).broadcast_to(
                [KB, NBLK, W_G])
            nc.vector.scalar_tensor_tensor(xm[:], locs_b, -1.0 / T, x_g[:],
                                           OP.mult, OP.add)

            # ---- c2 = xm^2 (rounded to f32r for the PE) ----
            c2 = mpool.tile([KB, NBLK, W_G], F32R, tag="c2")
            nc.scalar.activation(c2[:], xm[:], AT.Square)

            def c2s(k):
                return c2[:, k, :]

            # ---- P_last broadcast tile via weight-column matmuls ----
            plp = pp2.tile([KB, W_G], F32, tag="plp")
            nc.tensor.matmul(plp[:], lhsT(WA), c2s(NBLK - 1), start=True, stop=False)
            nc.tensor.matmul(plp[:], lhsT(WB), c2s(NBLK - 2), start=False, stop=False)
            nc.tensor.matmul(plp[:], lhsT(WC), c2s(NBLK - 3), start=False, stop=True)
            plb = lpool.tile([KB, W_G], F32, tag=f"plb{g}")
            nc.vector.tensor_copy(plb[:], plp[:])
            plb_tiles.append(plb)

            # ---- per half: scan matmuls -> c4s -> elementwise chain ----
            for hf in range(2):
                kofs = hf * HB
                c4s = mpool.tile([KB, HB, W_G], F32, tag="c4s")
                for w in range(2):
                    ps = ppool.tile([KB, 4, W_G], F32, tag="ps")
                    for kk in range(4):
                        blk = kofs + 4 * w + kk
                        cur = MA0 if blk == 0 else MA
                        nc.tensor.matmul(ps[:, kk, :], lhsT(cur), c2s(blk),
                                         start=True, stop=(blk == 0))
                        if blk >= 1:
                            prev = MB0 if blk == 1 else MB
                            nc.tensor.matmul(ps[:, kk, :], lhsT(prev),
                                             c2s(blk - 1),
                                             start=False, stop=(blk == 1))
                        if blk >= 2:
                            prev2 = MC0 if blk == 2 else MC
                            nc.tensor.matmul(ps[:, kk, :], lhsT(prev2),
                                             c2s(blk - 2),
                                             start=False, stop=True)
                    nc.vector.scalar_tensor_tensor(
                        c4s[:, 4 * w:4 * w + 4, :],
                        c2[:, kofs + 4 * w:kofs + 4 * w + 4, :].bitcast(F32),
                        4e-10, ps[:], OP.add, OP.max)

                xm_h = xm[:, kofs:kofs + HB, :]
                c2_h = c2[:, kofs:kofs + HB, :]
                rc = wpool1.tile([KB, HB, W_G], F32, tag="rc_q1")
                nc.vector.reciprocal_approx_fast(rc[:], c4s[:])
                rsq = wpool1.tile([KB, HB, W_G], F32, tag="rsq_vt")
                nc.scalar.activation(rsq[:], rc[:], AT.Sqrt)
                s_t = wpool1.tile([KB, HB, W_G], F32, tag="s_t")
                nc.vector.tensor_tensor(s_t[:], xm_h, rsq[:], OP.mult)
                om = wpool1.tile([KB, HB, W_G], F32, tag="om_q2")
                cdve(OMC_OP, om[:], c2_h.bitcast(F32), rc[:],
                     s0=3.0, s1=OMIN4)
                Lt = wpool1.tile([KB, HB, W_G], F32, tag="Lt")
                nc.scalar.activation(Lt[:], om[:], AT.Ln, scale=0.25 * (1.0 - 4e-5))
                vt = wpool1.tile([KB, HB, W_G], F32, tag="rsq_vt")
                nc.scalar.activation(vt[:], Lt[:], AT.Sqrt, scale=-1.0)
                q1 = wpool1.tile([KB, HB, W_G], F32, tag="rc_q1")
                cdve(H4_OP, q1[:], vt[:], g3t[:], s0=gc[0], s1=gc[1], imm2=gc[2])
                q2 = wpool1.tile([KB, HB, W_G], F32, tag="om_q2")
                cdve(H3S_OP, q2[:], q1[:], vt[:], s0=gc[4], s1=gc[5], imm2=gc[6])
                z_t = wpool2.tile([KB, HB, W_G], F32, tag="z_t")
                nc.vector.scalar_tensor_tensor(z_t[:], q2[:], gc[7], s_t[:],
                                               OP.add, OP.mult)
                for b in range(GB):
                    dst = out[b0 + b, kofs * KB:(kofs + HB) * KB, :].rearrange(
                        "(k p) d -> p k d", p=KB)
                    nc.sync.dma_start(dst, z_t[:, :, b * D:(b + 1) * D])

        # ================= forecast branch =================
        _zbn = [0]
        def zb(tag, pool=wpool1):
            _zbn[0] += 1
            return pool.tile([KB, NG, D], F32, tag=tag, name=f"zb{_zbn[0]}")

        zt = zb("s_t")
        for g in range(NG):
            for b in range(GB):
                nc.sync.dma_start(
                    zt[b * P_FUT:(b + 1) * P_FUT, g, :],
                    zf_in[g * GB + b])

        # tail path first (reuses zt-derived tiles early)
        x2 = zb("om_q2")
        nc.scalar.activation(x2[:], zt[:], AT.Square, scale=SQ12)
        xq = zb("rsq_vt")
        nc.scalar.activation(xq[:], zt[:], AT.Abs, scale=SQ12)
        ex = zb("rc_q1")
        nc.scalar.activation(ex[:], x2[:], AT.Exp, scale=-1.0)
        sp1 = zb("z_t", wpool2)
        cdve(H4_OP, sp1[:], xq[:], s3t[:], s0=sc[0], s1=sc[1], imm2=sc[2])
        sp2 = zb("acc8", wpool2)
        cdve(H3S_OP, sp2[:], sp1[:], xq[:], s0=sc[4], s1=sc[5], imm2=sc[6])
        wt = zb("z_t", wpool2)
        nc.vector.scalar_tensor_tensor(wt[:], sp2[:], sc[7], ex[:],
                                       OP.add, OP.mult)
        wc = zb("rsq_vt")
        nc.vector.tensor_scalar(wc[:], wt[:], 1e-6, None, OP.max)
        wm4 = zb("rc_q1")
        nc.vector.tensor_scalar(wm4[:], wc[:], -4.0, 4.0, OP.mult, OP.add)
        at_ = zb("acc8", wpool2)
        nc.vector.tensor_tensor(at_[:], wc[:], wm4[:], OP.mult)
        mask = zb("z_t", wpool2)
        nc.vector.tensor_scalar(mask[:], x2[:], 4.5, None, OP.is_gt)
        # central path
        e_t = zb("rc_q1")
        nc.scalar.activation(e_t[:], zt[:], AT.Erf, scale=SQ12)
        CLIP = 1.0 - 2e-6
        ec = zb("Lt")
        nc.vector.tensor_scalar(ec[:], e_t[:], CLIP, -CLIP, OP.min, OP.max)
        onem = zb("rsq_vt")
        nc.vector.tensor_scalar(onem[:], ec[:], -1.0, 1.0, OP.mult, OP.add)
        onep = zb("om_q2")
        nc.vector.tensor_scalar(onep[:], ec[:], 1.0, 1.0, OP.mult, OP.add)
        a0 = zb("rc_q1")
        nc.vector.tensor_tensor(a0[:], onem[:], onep[:], OP.mult)
        # blend: a = a0 + mask*(at - a0)
        dd = zb("om_q2")
        nc.vector.tensor_tensor(dd[:], at_[:], a0[:], OP.subtract)
        md = zb("rsq_vt")
        nc.vector.tensor_tensor(md[:], mask[:], dd[:], OP.mult)
        a_t = zb("s_t")
        nc.vector.tensor_tensor(a_t[:], a0[:], md[:], OP.add)
        ra = zb("rsq_vt")
        nc.vector.reciprocal_approx_fast(ra[:], a_t[:])
        rsa0 = zb("om_q2")
        nc.scalar.activation(rsa0[:], ra[:], AT.Sqrt)
        rsa = zb("z_t", wpool2)
        cdve(RNR_OP, rsa[:], rsa0[:], a_t[:], s0=1.5, s1=0.5)
        ae = zb("rc_q1")
        nc.scalar.activation(ae[:], ec[:], AT.Abs)
        ratio = zb("acc8", wpool2)
        nc.vector.tensor_tensor(ratio[:], ae[:], rsa[:], OP.mult)
        th = zb("om_q2")
        nc.scalar.activation(th[:], ratio[:], AT.Arctan)
        y_t = zb("rsq_vt")
        nc.scalar.activation(y_t[:], th[:], AT.Sin, scale=1.0 / 3.0,
                             bias=pio2[:])
        ym = zb("rc_q1")
        nc.vector.tensor_tensor(ym[:], y_t[:], rsa[:], OP.mult)
        inn = zb("om_q2")
        nc.vector.tensor_scalar(inn[:], ym[:], -1.0, 0.0, OP.add, OP.max)
        ip = zb("acc8", wpool2)
        for g in range(NG):
            for b in range(GB):
                nc.vector.tensor_tensor(
                    ip[b * P_FUT:(b + 1) * P_FUT, g, :],
                    inn[b * P_FUT:(b + 1) * P_FUT, g, :],
                    plb_tiles[g][b * P_FUT:(b + 1) * P_FUT, b * D:(b + 1) * D],
                    OP.mult)
        r_t = zb("rsq_vt")
        nc.scalar.activation(r_t[:], ip[:], AT.Sqrt)
        sg = zb("om_q2")
        nc.scalar.activation(sg[:], ec[:], AT.Sign)
        rs = zb("rc_q1")
        nc.vector.tensor_tensor(rs[:], r_t[:], sg[:], OP.mult)
        zo = zb("z_t", wpool2)
        for g in range(NG):
            for b in range(GB):
                nc.vector.scalar_tensor_tensor(
                    zo[b * P_FUT:(b + 1) * P_FUT, g, :],
                    locs_tiles[g][b * P_FUT:(b + 1) * P_FUT, b * D:(b + 1) * D],
                    1.0 / T,
                    rs[b * P_FUT:(b + 1) * P_FUT, g, :],
                    OP.mult, OP.add)
        for g in range(NG):
            for b in range(GB):
                nc.sync.dma_start(out[g * GB + b, T:T + P_FUT, :],
                                  zo[b * P_FUT:(b + 1) * P_FUT, g, :])

    nc.compile()
    _CACHE["nc"] = nc
    return nc


def kernel(x_hist, z_future):
    from concourse.bass_utils import run_bass_kernel_spmd
    nc = _build_program()
    mats = _build_matrices()
    x_hist = np.ascontiguousarray(x_hist, dtype=np.float32)
    z_future = np.ascontiguousarray(z_future, dtype=np.float32)
    in_maps = []
    for c in range(NCORES):
        sl = slice(c * B_CORE, (c + 1) * B_CORE)
        in_maps.append({"x": x_hist[sl], "zf": z_future[sl], "mats": mats})
    res = run_bass_kernel_spmd(nc, in_maps, core_ids=list(range(NCORES)))
    return np.concatenate([r["out"] for r in res.results], axis=0)

